# revision 23
# baseline (speedup 1.0000x reference)
"""Trainium2 Bass kernel for the KeypointLoss problem.

Full inputs:
  combined_preds [16, 4, 22, 128, 128] f32
  heatmaps       [16, 11, 128, 128]    f32
  labels         [16, 11, 11]          f32
Outputs (matching the reference):
  heat_loss  [16, 4] f32
  label_loss [16, 4] f32

Sharding: pure data parallel over the batch dim — core i handles batches
[2i, 2i+2). Each core computes its [2, 4] slices of both losses.

Per-core layout (B_L=2, S=4, K=11, H=W=128; G = B_L*S = 8 groups,
PL = G*K = 88 planes). All big tiles are h-major: [h=128 partitions,
(k, w) free].

HW-measured DMA reality (this session's dmabench): the 512B-chunk
h-major load pattern runs at ~270 GB/s/core aggregate REGARDLESS of ring
count (1 ring, 2 rings, 3 queues incl SWDGE all ~26.6us for 7.2MB).
A fully contiguous chop pattern reaches ~321 GB/s but scrambles the
per-plane structure the argmax needs. So the loads are the ~26.6us
floor here; the kernel's job is to hide ALL compute under them and keep
the post-load tail minimal.

Compute structure (v1):
  heat_loss[b,s] = sum_khw d^2 where d = hm - ht computed directly as
  bf16 (one DVE/GPSIMD tensor_tensor per group) and squared-summed on
  PE via self-matmuls accumulated in PSUM; per-group diagonals extracted
  with DVE stt+accum. This removes the old ScalarE cast+square serial
  chain (~14.5us) entirely — ScalarE only hosts its DMA ring + two tiny
  ops (cls, conf squares).

  label_loss: rowmax R per group (DVE; groups 6/7 on GPSIMD, split into
  k-halves following the split final DMAs), argmax head per 2-group
  chunk (PE transpose -> Mv -> x -> ridu) so only the LAST chunk's head
  trails the final load; one indirect-DMA gather (SWDGE) of the argmax
  rows; y extract; fused stt tail. Small inputs (aux, labels, pred9)
  ride SWDGE so they never delay the rings.
"""

import os as _os
import sys

for _p in ("/opt/trn_rl_repo", "/root/.axon_site/_ro/trn_rl_repo"):
    if _p not in sys.path:
        sys.path.append(_p)

from contextlib import ExitStack

import numpy as np

# Problem constants (hardcoded per the task contract).
B, S, K, H, W = 16, 4, 11, 128, 128
NCORES = 8
BL = B // NCORES          # local batch per core = 2
G = BL * S                # groups per core = 8
PL = G * K                # planes per core = 88
KW = K * W                # free size of one group tile = 1408
C2 = 2 * K                # channel count of combined_preds = 22
KSPLITS = [(0, 4), (4, 8), (8, 11)]   # k-splits of the final group loads

_CACHE = {}


def _build_module(reps=1, loop_n=1):
    import concourse.bass as bass
    import concourse.tile as tile
    from concourse import bacc, mybir

    f32 = mybir.dt.float32
    bf16 = mybir.dt.bfloat16
    Alu = mybir.AluOpType
    Act = mybir.ActivationFunctionType
    Ax = mybir.AxisListType

    nc = bacc.Bacc("TRN2", debug=False, enable_asserts=False, num_devices=1)

    cp = nc.dram_tensor("cp", [BL, S, C2, H, W], f32, kind="ExternalInput").ap()
    hmr = nc.dram_tensor("hmr", [BL, K, H, W], f32, kind="ExternalInput").ap()
    lbl = nc.dram_tensor("lbl", [BL, K, 11], f32, kind="ExternalInput").ap()
    out_all = nc.dram_tensor("out_all", [1, 2 * G], f32, kind="ExternalOutput").ap()

    # Inline constants packed into ONE tensor -> one aux DMA.
    # cols [0,128) identity, [128,256) iota, 256 ones, 257 rbase, [258,266) kmap
    aux_np = np.zeros((128, 269), np.float32)
    aux_np[:, 0:128] = np.eye(128, dtype=np.float32)
    aux_np[:, 128:256] = np.arange(128, dtype=np.float32)[None, :]
    aux_np[:, 256] = 1.0
    # rbase: DRAM row index (in units of W-element rows) of (plane, h=0)
    # within cp viewed as [(BL*S*C2*H), W]. Stored per 32-plane head chunk
    # (cols 257+c, rows 0..chunk) so chunk ops stay at base partition 0.
    for g in range(G):
        b, s = divmod(g, S)
        for k in range(K):
            pl = g * K + k
            aux_np[pl % 32, 257 + pl // 32] = ((b * S + s) * C2 + k) * H
            aux_np[pl, 261 + g] = 1.0  # plane->group indicator
    aux_c = nc.inline_tensor(aux_np, "auxc").ap()

    with tile.TileContext(nc) as tc, ExitStack() as ctx:
        bufs = 1 if reps == 1 else 2
        sb = ctx.enter_context(tc.tile_pool(name="sb", bufs=bufs))
        scr = ctx.enter_context(tc.tile_pool(name="scr", bufs=2))
        ps = ctx.enter_context(tc.tile_pool(name="ps", bufs=1, space="PSUM"))

        def emit():
            # ---- small loads on SWDGE (gpsimd) so the HWDGE rings carry
            # only the big tiles ----
            aux_t = sb.tile([128, 269], f32, name="aux_t")
            nc.gpsimd.dma_start(aux_t[:], aux_c)
            id_t = aux_t[:, 0:128]
            io_t = aux_t[:, 128:256]
            on_t = aux_t[:, 256:257]
            km_t = aux_t[0:PL, 261:269]

            APc = type(lbl)
            lblr = sb.tile([PL, 11], f32, name="lblr")
            for b in range(BL):
                src_b = APc(lbl.tensor, b * K * 11, [[0, S], [11, K], [1, 11]])
                nc.gpsimd.dma_start(lblr[b * S * K : (b + 1) * S * K, :], src_b)
            pred9 = sb.tile([PL, 9], f32, name="pred9")
            nc.gpsimd.dma_start(pred9[:], cp[:, :, K:C2, 0, 0:9])

            # zero row for the bank-clearing matmuls
            zrow = sb.tile([1, 512], bf16, name="zrow")
            nc.gpsimd.memset(zrow[:], 0.0)

            # ---- big loads: 2 HWDGE rings; final groups split in k so
            # their rowmax/subtract can start before the last bytes ----
            ht_ts = []
            for b in range(BL):
                ht_t = sb.tile([128, KW], f32, name=f"ht{b}")
                eng = nc.sync if b == 0 else nc.scalar
                eng.dma_start(
                    ht_t[:].rearrange("h (k w) -> h k w", k=K),
                    hmr[b].rearrange("k h w -> h k w"),
                )
                ht_ts.append(ht_t)
            hm_big = sb.tile([128, G * KW], f32, name="hm_big")

            def hm_g(g):
                return hm_big[:, g * KW : (g + 1) * KW]

            for g in range(6):
                b, s = divmod(g, S)
                eng = nc.sync if g % 2 == 0 else nc.scalar
                eng.dma_start(
                    hm_g(g).rearrange("h (k w) -> h k w", k=K),
                    cp[b, s, 0:K].rearrange("k h w -> h k w"),
                )
            for k0, k1 in KSPLITS:
                for g, eng in ((6, nc.sync), (7, nc.scalar)):
                    b, s = divmod(g, S)
                    eng.dma_start(
                        hm_g(g)[:, k0 * W : k1 * W].rearrange(
                            "h (k w) -> h k w", k=k1 - k0
                        ),
                        cp[b, s, k0:k1].rearrange("k h w -> h k w"),
                    )

            # ---- early label prep (only needs lblr/pred9) ----
            cdiff = sb.tile([PL, 7], f32, name="cdiff")
            nc.vector.tensor_tensor(
                out=cdiff[:], in0=pred9[:, 0:7], in1=lblr[:, 0:7], op=Alu.subtract
            )
            csc = sb.tile([PL, 7], f32, name="csc")
            cls = sb.tile([PL, 1], f32, name="cls")
            nc.scalar.activation(
                out=csc[:], in_=cdiff[:], func=Act.Square, accum_out=cls[:]
            )
            t1 = sb.tile([PL, 1], f32, name="t1")
            nc.vector.tensor_tensor(t1[:], lblr[:, 9:10], lblr[:, 7:8], Alu.add)
            t3 = sb.tile([PL, 1], f32, name="t3")
            nc.vector.tensor_tensor(t3[:], lblr[:, 10:11], lblr[:, 8:9], Alu.add)
            gmin = sb.tile([PL, 1], f32, name="gmin")
            nc.vector.tensor_tensor(gmin[:], lblr[:, 9:10], lblr[:, 10:11], Alu.min)
            gmax = sb.tile([PL, 1], f32, name="gmax")
            nc.vector.tensor_tensor(gmax[:], lblr[:, 9:10], lblr[:, 10:11], Alu.max)
            c1 = sb.tile([PL, 1], f32, name="c1")
            nc.vector.tensor_scalar(c1[:], gmin[:], 0.0, None, Alu.is_gt)
            c2t = sb.tile([PL, 1], f32, name="c2t")
            nc.vector.tensor_scalar(c2t[:], gmax[:], float(H), None, Alu.is_lt)
            vv = sb.tile([PL, 1], f32, name="vv")
            nc.vector.tensor_tensor(vv[:], c1[:], c2t[:], Alu.mult)

            # ---- PSUM banks for the d self-matmuls, pre-cleared ----
            psq = []
            for half in range(2):
                p = ps.tile([128, 512], f32, name=f"psq{half}", tag=f"psq{half}")
                nc.tensor.matmul(
                    out=p[:], lhsT=zrow[:, 0:128], rhs=zrow[:],
                    start=True, stop=False, skip_group_check=True,
                )
                psq.append(p)

            def psq_g(g):
                return psq[g // 4][:, (g % 4) * 128 : (g % 4 + 1) * 128]

            d_big = sb.tile([128, G * KW], bf16, name="d_big")

            def d_g(g):
                return d_big[:, g * KW : (g + 1) * KW]

            def sub_g(g, eng, k0=0, k1=K):
                b = g // S
                eng.tensor_tensor(
                    out=d_g(g)[:, k0 * W : k1 * W],
                    in0=hm_g(g)[:, k0 * W : k1 * W],
                    in1=ht_ts[b][:, k0 * W : k1 * W],
                    op=Alu.subtract,
                )

            def sq_g(g, k0=0, k1=K):
                for k in range(k0, k1):
                    nc.tensor.matmul(
                        out=psq_g(g),
                        lhsT=d_g(g)[:, k * 128 : (k + 1) * 128],
                        rhs=d_g(g)[:, k * 128 : (k + 1) * 128],
                        start=False,
                        stop=(k == K - 1),
                        skip_group_check=True,
                    )

            # ---- per-group rowmax + subtract for groups 0..5 ----
            R_all = sb.tile([128, PL], f32, name="R_all")

            def rowmax(g, eng, k0=0, k1=K):
                eng.tensor_reduce(
                    out=R_all[:, g * K + k0 : g * K + k1],
                    in_=hm_g(g)[:, k0 * W : k1 * W].rearrange(
                        "h (k w) -> h k w", k=k1 - k0
                    ),
                    axis=Ax.X,
                    op=Alu.max,
                )

            hsum = sb.tile([128, G], f32, name="hsum")
            Mv = sb.tile([PL, 1], f32, name="Mv")
            xf = sb.tile([PL, 1], f32, name="xf")
            ridu = sb.tile([PL, 1], mybir.dt.uint32, name="ridu")
            # transpose outputs must land at PSUM partition 0 (walrus rule)
            # -> one PSUM bank per 32-plane head chunk
            HCHUNKS = [(0, 32), (32, 64), (64, 88)]
            psum_rt = [
                ps.tile([hi - lo, 128], f32, name=f"psum_rt{c}", tag=f"rt{c}")
                for c, (lo, hi) in enumerate(HCHUNKS)
            ]

            def diag(g):
                dsc = scr.tile([128, 128], f32, name=f"dsc{g}", tag="dsc")
                return nc.vector.scalar_tensor_tensor(
                    out=dsc[:],
                    in0=psq_g(g),
                    scalar=1.0,
                    in1=id_t[:],
                    op0=Alu.bypass,
                    op1=Alu.mult,
                    accum_out=hsum[:, g : g + 1],
                )

            def head_chunk(c):
                # argmax head for plane chunk c. All chunk math runs at base
                # partition 0 (walrus: SB inputs must share base partition);
                # results are copied into the full [PL] columns afterwards.
                lo, hi = HCHUNKS[c]
                n = hi - lo
                rt = psum_rt[c][:]
                nc.tensor.transpose(
                    out=rt, in_=R_all[:, lo:hi], identity=id_t[:]
                )
                mvc = scr.tile([32, 1], f32, name=f"mvc{c}", tag="mvc")
                nc.vector.tensor_reduce(
                    out=mvc[0:n], in_=rt, axis=Ax.X, op=Alu.max
                )
                nc.vector.tensor_copy(out=Mv[lo:hi], in_=mvc[0:n])
                xsc = scr.tile([32, 128], f32, name=f"xsc{c}", tag="xysc")
                xfc = scr.tile([32, 1], f32, name=f"xfc{c}", tag="xfc")
                nc.vector.scalar_tensor_tensor(
                    out=xsc[0:n, :], in0=rt, scalar=mvc[0:n, 0:1],
                    in1=io_t[0:n, :], op0=Alu.is_equal, op1=Alu.mult,
                    accum_out=xfc[0:n],
                )
                nc.vector.tensor_copy(out=xf[lo:hi], in_=xfc[0:n])
                rdc = scr.tile([32, 1], mybir.dt.uint32, name=f"rdc{c}", tag="rdc")
                nc.vector.tensor_tensor(
                    out=rdc[0:n], in0=xfc[0:n],
                    in1=aux_t[0:n, 257 + c : 258 + c], op=Alu.add
                )
                return nc.vector.tensor_copy(out=ridu[lo:hi], in_=rdc[0:n])

            # groups 0..5: rowmax on DVE, subtract per SUBENG config
            sub_cfg = _os.environ.get("SUBENG", "pppppppp")
            skips = set(_os.environ.get("KSKIP", "").split(","))
            sub_eng = {
                "p": nc.gpsimd, "v": nc.vector
            }

            if "sub" in skips:
                def sub_g(g, eng, k0=0, k1=K):  # noqa: F811
                    pass
                def sq_g(g, k0=0, k1=K):  # noqa: F811
                    pass
                def diag(g):  # noqa: F811
                    return nc.vector.memset(hsum[:, g : g + 1], 0.0)
            if "head" in skips:
                def rowmax(g, eng, k0=0, k1=K):  # noqa: F811
                    pass
                def head_chunk(c):  # noqa: F811
                    lo, hi = HCHUNKS[c]
                    nc.vector.memset(Mv[lo:hi], 0.0)
                    nc.vector.memset(xf[lo:hi], 0.0)
                    return nc.vector.memset(ridu[lo:hi], 0)

            for g in range(6):
                rowmax(g, nc.vector)
                sub_g(g, sub_eng[sub_cfg[g]])
                sq_g(g)
                if g == 2:
                    head_chunk(0)   # planes 0..31 complete after g2 rowmax
                elif g == 5:
                    head_chunk(1)   # planes 32..63 complete after g5 rowmax
                diag(g)

            # groups 6/7: follow the split loads; rowmax on DVE (GPSIMD has
            # no free-axis reduce), subtract on GPSIMD
            for k0, k1 in KSPLITS:
                rowmax(6, nc.vector, k0, k1)
                rowmax(7, nc.vector, k0, k1)
                sub_g(6, sub_eng[sub_cfg[6]], k0, k1)
                sq_g(6, k0, k1)
                sub_g(7, sub_eng[sub_cfg[7]], k0, k1)
                sq_g(7, k0, k1)
            ridu_inst = head_chunk(2)   # planes 64..87

            gath = sb.tile([PL, 128], f32, name="gath")
            nc.gpsimd.indirect_dma_start(
                out=gath[:],
                out_offset=None,
                in_=cp.rearrange("b s c h w -> (b s c h) w"),
                in_offset=bass.IndirectOffsetOnAxis(ap=ridu[:, 0:1], axis=0),
            )

            # x-side tail ops (need xf only) before the diags
            u = sb.tile([PL, 1], f32, name="u")
            nc.vector.scalar_tensor_tensor(
                out=u[:], in0=xf[:], scalar=pred9[:, 7:8], in1=t1[:],
                op0=Alu.add, op1=Alu.subtract,
            )
            w1 = sb.tile([PL, 1], f32, name="w1")
            nc.vector.scalar_tensor_tensor(
                out=w1[:], in0=u[:], scalar=u[:, 0:1], in1=cls[:],
                op0=Alu.mult, op1=Alu.add,
            )

            # fill the gather window with the last two diags
            d6 = diag(6)
            d7 = diag(7)
            tile.add_dep_helper(
                d6.ins, ridu_inst.ins, sync=False, reason="defer diag past head"
            )

            # ---- y from the gathered rows ----
            ysc = scr.tile([PL, 128], f32, name="ysc", tag="xysc")
            yf = sb.tile([PL, 1], f32, name="yf")
            nc.vector.scalar_tensor_tensor(
                out=ysc[:], in0=gath[:], scalar=Mv[:, 0:1], in1=io_t[0:PL, :],
                op0=Alu.is_equal, op1=Alu.mult, accum_out=yf[:],
            )

            conf = sb.tile([PL, 1], f32, name="conf")
            nc.scalar.activation(
                out=conf[:], in_=Mv[:], func=Act.Square, bias=1.0, scale=-1.0
            )

            # v = (yf+p8)-t3; w2 = v*v + w1; perkp = (w2+conf)*vv
            v = sb.tile([PL, 1], f32, name="v")
            nc.vector.scalar_tensor_tensor(
                out=v[:], in0=yf[:], scalar=pred9[:, 8:9], in1=t3[:],
                op0=Alu.add, op1=Alu.subtract,
            )
            w2 = sb.tile([PL, 1], f32, name="w2")
            nc.vector.scalar_tensor_tensor(
                out=w2[:], in0=v[:], scalar=v[:, 0:1], in1=w1[:],
                op0=Alu.mult, op1=Alu.add,
            )
            perkp = sb.tile([PL, 1], f32, name="perkp")
            nc.vector.scalar_tensor_tensor(
                out=perkp[:], in0=w2[:], scalar=conf[:, 0:1], in1=vv[:],
                op0=Alu.add, op1=Alu.mult,
            )

            # ---- final reductions into ONE PSUM bank, single out DMA ----
            psum_out = ps.tile([1, 2 * G], f32, name="psum_out", tag="fin")
            nc.tensor.matmul(
                out=psum_out[:, 0:G], lhsT=on_t[:], rhs=hsum[:],
                start=True, stop=False, skip_group_check=True,
            )
            nc.tensor.matmul(
                out=psum_out[:, G : 2 * G], lhsT=perkp[:], rhs=km_t[:],
                start=False, stop=True, skip_group_check=True,
            )
            out_row = sb.tile([1, 2 * G], f32, name="out_row")
            nc.vector.tensor_copy(out=out_row[:], in_=psum_out[:])
            nc.sync.dma_start(out_all, out_row[:])

        if loop_n > 1:
            # on-device timing loop: each iteration is separated by the
            # For_i back-edge barrier, so wall time ~= N * (span + ~2us)
            with tc.For_i(0, loop_n, 1):
                emit()
        else:
            for _ in range(reps):
                emit()

    nc.compile()
    return nc


def _get_nc(reps=1, loop_n=1):
    key = f"nc{reps}_{loop_n}"
    if key not in _CACHE:
        _CACHE[key] = _build_module(reps, loop_n)
    return _CACHE[key]


def _in_maps(combined_preds, heatmaps, labels):
    cp = np.ascontiguousarray(combined_preds, dtype=np.float32)
    hmr = np.ascontiguousarray(heatmaps, dtype=np.float32)
    lb = np.ascontiguousarray(labels, dtype=np.float32)
    maps = []
    for i in range(NCORES):
        b0 = BL * i
        maps.append(
            {
                "cp": np.ascontiguousarray(cp[b0 : b0 + BL]),
                "hmr": np.ascontiguousarray(hmr[b0 : b0 + BL]),
                "lbl": np.ascontiguousarray(lb[b0 : b0 + BL]),
            }
        )
    return maps


def run(combined_preds, heatmaps, labels, trace=False):
    """Run on hardware; returns ((heat, label), BassKernelResults)."""
    from concourse import bass_utils

    nc = _get_nc()
    res = bass_utils.run_bass_kernel_spmd(
        nc,
        _in_maps(combined_preds, heatmaps, labels),
        core_ids=list(range(NCORES)),
        trace=trace,
    )
    heat = np.concatenate(
        [res.results[i]["out_all"][:, 0:G].reshape(BL, S) for i in range(NCORES)],
        axis=0,
    )
    lab = np.concatenate(
        [res.results[i]["out_all"][:, G : 2 * G].reshape(BL, S) for i in range(NCORES)],
        axis=0,
    )
    return (heat, lab), res


def kernel(combined_preds, heatmaps, labels):
    (heat, lab), _ = run(combined_preds, heatmaps, labels)
    return heat, lab


# revision 25
# speedup vs baseline: 1.2079x; 1.2079x over previous
"""Trainium2 Bass kernel for the KeypointLoss problem.

Full inputs:
  combined_preds [16, 4, 22, 128, 128] f32
  heatmaps       [16, 11, 128, 128]    f32
  labels         [16, 11, 11]          f32
Outputs (matching the reference):
  heat_loss  [16, 4] f32
  label_loss [16, 4] f32

Sharding: pure data parallel over the batch dim — core i handles batches
[2i, 2i+2). Each core computes its [2, 4] slices of both losses.

Per-core layout (B_L=2, S=4, K=11, H=W=128; G = B_L*S = 8 groups,
PL = G*K = 88 planes). All big tiles are h-major: [h=128 partitions,
(k, w) free].

HW-measured DMA reality (this session's dmabench): the 512B-chunk
h-major load pattern runs at ~270 GB/s/core aggregate REGARDLESS of ring
count (1 ring, 2 rings, 3 queues incl SWDGE all ~26.6us for 7.2MB).
A fully contiguous chop pattern reaches ~321 GB/s but scrambles the
per-plane structure the argmax needs. So the loads are the ~26.6us
floor here; the kernel's job is to hide ALL compute under them and keep
the post-load tail minimal.

Compute structure (v1):
  heat_loss[b,s] = sum_khw d^2 where d = hm - ht computed directly as
  bf16 (one DVE/GPSIMD tensor_tensor per group) and squared-summed on
  PE via self-matmuls accumulated in PSUM; per-group diagonals extracted
  with DVE stt+accum. This removes the old ScalarE cast+square serial
  chain (~14.5us) entirely — ScalarE only hosts its DMA ring + two tiny
  ops (cls, conf squares).

  label_loss: rowmax R per group (DVE; groups 6/7 on GPSIMD, split into
  k-halves following the split final DMAs), argmax head per 2-group
  chunk (PE transpose -> Mv -> x -> ridu) so only the LAST chunk's head
  trails the final load; one indirect-DMA gather (SWDGE) of the argmax
  rows; y extract; fused stt tail. Small inputs (aux, labels, pred9)
  ride SWDGE so they never delay the rings.
"""

import os as _os
import sys

for _p in ("/opt/trn_rl_repo", "/root/.axon_site/_ro/trn_rl_repo"):
    if _p not in sys.path:
        sys.path.append(_p)

from contextlib import ExitStack

import numpy as np

# Problem constants (hardcoded per the task contract).
B, S, K, H, W = 16, 4, 11, 128, 128
NCORES = 8
BL = B // NCORES          # local batch per core = 2
G = BL * S                # groups per core = 8
PL = G * K                # planes per core = 88
KW = K * W                # free size of one group tile = 1408
C2 = 2 * K                # channel count of combined_preds = 22
KSPLITS = [(0, 4), (4, 8), (8, 11)]   # k-splits of the final group loads

_CACHE = {}


def _build_module(reps=1, loop_n=1):
    import concourse.bass as bass
    import concourse.tile as tile
    from concourse import bacc, mybir

    f32 = mybir.dt.float32
    bf16 = mybir.dt.bfloat16
    Alu = mybir.AluOpType
    Act = mybir.ActivationFunctionType
    Ax = mybir.AxisListType

    nc = bacc.Bacc("TRN2", debug=False, enable_asserts=False, num_devices=1)

    cp = nc.dram_tensor("cp", [BL, S, C2, H, W], f32, kind="ExternalInput").ap()
    hmr = nc.dram_tensor("hmr", [BL, K, H, W], f32, kind="ExternalInput").ap()
    lbl = nc.dram_tensor("lbl", [BL, K, 11], f32, kind="ExternalInput").ap()
    out_all = nc.dram_tensor("out_all", [1, 2 * G], f32, kind="ExternalOutput").ap()

    # Inline constants packed into ONE tensor -> one aux DMA.
    # cols [0,128) identity, [128,256) iota, 256 ones, 257 rbase, [258,266) kmap
    aux_np = np.zeros((128, 269), np.float32)
    aux_np[:, 0:128] = np.eye(128, dtype=np.float32)
    aux_np[:, 128:256] = np.arange(128, dtype=np.float32)[None, :]
    aux_np[:, 256] = 1.0
    # rbase: DRAM row index (in units of W-element rows) of (plane, h=0)
    # within cp viewed as [(BL*S*C2*H), W]. Stored per 32-plane head chunk
    # (cols 257+c, rows 0..chunk) so chunk ops stay at base partition 0.
    for g in range(G):
        b, s = divmod(g, S)
        for k in range(K):
            pl = g * K + k
            aux_np[pl % 32, 257 + pl // 32] = ((b * S + s) * C2 + k) * H
            aux_np[pl, 261 + g] = 1.0  # plane->group indicator
    aux_c = nc.inline_tensor(aux_np, "auxc").ap()

    with tile.TileContext(nc) as tc, ExitStack() as ctx:
        bufs = 1 if reps == 1 else 2
        sb = ctx.enter_context(tc.tile_pool(name="sb", bufs=bufs))
        scr = ctx.enter_context(tc.tile_pool(name="scr", bufs=2))
        ps = ctx.enter_context(tc.tile_pool(name="ps", bufs=1, space="PSUM"))

        def emit():
            # ---- small loads on SWDGE (gpsimd) so the HWDGE rings carry
            # only the big tiles ----
            aux_t = sb.tile([128, 269], f32, name="aux_t")
            nc.gpsimd.dma_start(aux_t[:], aux_c)
            id_t = aux_t[:, 0:128]
            io_t = aux_t[:, 128:256]
            on_t = aux_t[:, 256:257]
            km_t = aux_t[0:PL, 261:269]

            APc = type(lbl)
            lblr = sb.tile([PL, 11], f32, name="lblr")
            for b in range(BL):
                src_b = APc(lbl.tensor, b * K * 11, [[0, S], [11, K], [1, 11]])
                nc.gpsimd.dma_start(lblr[b * S * K : (b + 1) * S * K, :], src_b)
            pred9 = sb.tile([PL, 9], f32, name="pred9")
            nc.gpsimd.dma_start(pred9[:], cp[:, :, K:C2, 0, 0:9])

            # zero row for the bank-clearing matmuls
            zrow = sb.tile([1, 512], bf16, name="zrow")
            nc.gpsimd.memset(zrow[:], 0.0)

            # ---- big loads: 2 HWDGE rings; final groups split in k so
            # their rowmax/subtract can start before the last bytes ----
            ht_ts = []
            for b in range(BL):
                ht_t = sb.tile([128, KW], f32, name=f"ht{b}")
                eng = nc.sync if b == 0 else nc.scalar
                eng.dma_start(
                    ht_t[:].rearrange("h (k w) -> h k w", k=K),
                    hmr[b].rearrange("k h w -> h k w"),
                )
                ht_ts.append(ht_t)
            hm_big = sb.tile([128, G * KW], f32, name="hm_big")

            def hm_g(g):
                return hm_big[:, g * KW : (g + 1) * KW]

            for g in range(6):
                b, s = divmod(g, S)
                eng = nc.sync if g % 2 == 0 else nc.scalar
                eng.dma_start(
                    hm_g(g).rearrange("h (k w) -> h k w", k=K),
                    cp[b, s, 0:K].rearrange("k h w -> h k w"),
                )
            for k0, k1 in KSPLITS:
                for g, eng in ((6, nc.sync), (7, nc.scalar)):
                    b, s = divmod(g, S)
                    eng.dma_start(
                        hm_g(g)[:, k0 * W : k1 * W].rearrange(
                            "h (k w) -> h k w", k=k1 - k0
                        ),
                        cp[b, s, k0:k1].rearrange("k h w -> h k w"),
                    )

            # ---- early label prep (only needs lblr/pred9) ----
            cdiff = sb.tile([PL, 7], f32, name="cdiff")
            nc.vector.tensor_tensor(
                out=cdiff[:], in0=pred9[:, 0:7], in1=lblr[:, 0:7], op=Alu.subtract
            )
            csc = sb.tile([PL, 7], f32, name="csc")
            cls = sb.tile([PL, 1], f32, name="cls")
            nc.scalar.activation(
                out=csc[:], in_=cdiff[:], func=Act.Square, accum_out=cls[:]
            )
            t1 = sb.tile([PL, 1], f32, name="t1")
            nc.vector.tensor_tensor(t1[:], lblr[:, 9:10], lblr[:, 7:8], Alu.add)
            t3 = sb.tile([PL, 1], f32, name="t3")
            nc.vector.tensor_tensor(t3[:], lblr[:, 10:11], lblr[:, 8:9], Alu.add)
            gmin = sb.tile([PL, 1], f32, name="gmin")
            nc.vector.tensor_tensor(gmin[:], lblr[:, 9:10], lblr[:, 10:11], Alu.min)
            gmax = sb.tile([PL, 1], f32, name="gmax")
            nc.vector.tensor_tensor(gmax[:], lblr[:, 9:10], lblr[:, 10:11], Alu.max)
            c1 = sb.tile([PL, 1], f32, name="c1")
            nc.vector.tensor_scalar(c1[:], gmin[:], 0.0, None, Alu.is_gt)
            c2t = sb.tile([PL, 1], f32, name="c2t")
            nc.vector.tensor_scalar(c2t[:], gmax[:], float(H), None, Alu.is_lt)
            vv = sb.tile([PL, 1], f32, name="vv")
            nc.vector.tensor_tensor(vv[:], c1[:], c2t[:], Alu.mult)

            # ---- PSUM banks for the d self-matmuls, pre-cleared ----
            psq = []
            for half in range(2):
                p = ps.tile([128, 512], f32, name=f"psq{half}", tag=f"psq{half}")
                nc.tensor.matmul(
                    out=p[:], lhsT=zrow[:, 0:128], rhs=zrow[:],
                    start=True, stop=False, skip_group_check=True,
                )
                psq.append(p)

            def psq_g(g):
                return psq[g // 4][:, (g % 4) * 128 : (g % 4 + 1) * 128]

            d_big = sb.tile([128, G * KW], bf16, name="d_big")

            def d_g(g):
                return d_big[:, g * KW : (g + 1) * KW]

            def sub_g(g, eng, k0=0, k1=K):
                b = g // S
                eng.tensor_tensor(
                    out=d_g(g)[:, k0 * W : k1 * W],
                    in0=hm_g(g)[:, k0 * W : k1 * W],
                    in1=ht_ts[b][:, k0 * W : k1 * W],
                    op=Alu.subtract,
                )

            def sq_g(g, k0=0, k1=K):
                for k in range(k0, k1):
                    nc.tensor.matmul(
                        out=psq_g(g),
                        lhsT=d_g(g)[:, k * 128 : (k + 1) * 128],
                        rhs=d_g(g)[:, k * 128 : (k + 1) * 128],
                        start=False,
                        stop=(k == K - 1),
                        skip_group_check=True,
                    )

            # ---- per-group rowmax + subtract for groups 0..5 ----
            R_all = sb.tile([128, PL], f32, name="R_all")

            def rowmax(g, eng, k0=0, k1=K):
                eng.tensor_reduce(
                    out=R_all[:, g * K + k0 : g * K + k1],
                    in_=hm_g(g)[:, k0 * W : k1 * W].rearrange(
                        "h (k w) -> h k w", k=k1 - k0
                    ),
                    axis=Ax.X,
                    op=Alu.max,
                )

            hsum = sb.tile([128, G], f32, name="hsum")
            Mv = sb.tile([PL, 1], f32, name="Mv")
            xf = sb.tile([PL, 1], f32, name="xf")
            ridu = sb.tile([PL, 1], mybir.dt.uint32, name="ridu")
            # transpose outputs must land at PSUM partition 0 (walrus rule)
            # -> one PSUM bank per 32-plane head chunk
            HCHUNKS = [(0, 32), (32, 64), (64, 88)]
            psum_rt = [
                ps.tile([hi - lo, 128], f32, name=f"psum_rt{c}", tag=f"rt{c}")
                for c, (lo, hi) in enumerate(HCHUNKS)
            ]

            def diag(g):
                dsc = scr.tile([128, 128], f32, name=f"dsc{g}", tag="dsc")
                return nc.vector.scalar_tensor_tensor(
                    out=dsc[:],
                    in0=psq_g(g),
                    scalar=1.0,
                    in1=id_t[:],
                    op0=Alu.bypass,
                    op1=Alu.mult,
                    accum_out=hsum[:, g : g + 1],
                )

            def head_chunk(c):
                # argmax head for plane chunk c. All chunk math runs at base
                # partition 0 (walrus: SB inputs must share base partition);
                # results are copied into the full [PL] columns afterwards.
                lo, hi = HCHUNKS[c]
                n = hi - lo
                rt = psum_rt[c][:]
                nc.tensor.transpose(
                    out=rt, in_=R_all[:, lo:hi], identity=id_t[:]
                )
                mvc = scr.tile([32, 1], f32, name=f"mvc{c}", tag="mvc")
                nc.vector.tensor_reduce(
                    out=mvc[0:n], in_=rt, axis=Ax.X, op=Alu.max
                )
                nc.vector.tensor_copy(out=Mv[lo:hi], in_=mvc[0:n])
                xsc = scr.tile([32, 128], f32, name=f"xsc{c}", tag="xysc")
                xfc = scr.tile([32, 1], f32, name=f"xfc{c}", tag="xfc")
                nc.vector.scalar_tensor_tensor(
                    out=xsc[0:n, :], in0=rt, scalar=mvc[0:n, 0:1],
                    in1=io_t[0:n, :], op0=Alu.is_equal, op1=Alu.mult,
                    accum_out=xfc[0:n],
                )
                nc.vector.tensor_copy(out=xf[lo:hi], in_=xfc[0:n])
                rdc = scr.tile([32, 1], mybir.dt.uint32, name=f"rdc{c}", tag="rdc")
                nc.vector.tensor_tensor(
                    out=rdc[0:n], in0=xfc[0:n],
                    in1=aux_t[0:n, 257 + c : 258 + c], op=Alu.add
                )
                return nc.vector.tensor_copy(out=ridu[lo:hi], in_=rdc[0:n])

            # groups 0..5: rowmax on DVE, subtract per SUBENG config
            sub_cfg = _os.environ.get("SUBENG", "pppppppp")
            skips = set(_os.environ.get("KSKIP", "").split(","))
            sub_eng = {
                "p": nc.gpsimd, "v": nc.vector
            }

            if "sub" in skips:
                def sub_g(g, eng, k0=0, k1=K):  # noqa: F811
                    pass
                def sq_g(g, k0=0, k1=K):  # noqa: F811
                    pass
                def diag(g):  # noqa: F811
                    return nc.vector.memset(hsum[:, g : g + 1], 0.0)
            if "head" in skips or "rowmax" in skips:
                def rowmax(g, eng, k0=0, k1=K):  # noqa: F811
                    pass
            if "head" in skips or "xhead" in skips:
                def head_chunk(c):  # noqa: F811
                    lo, hi = HCHUNKS[c]
                    nc.vector.memset(Mv[lo:hi], 0.0)
                    nc.vector.memset(xf[lo:hi], 0.0)
                    return nc.vector.memset(ridu[lo:hi], 0)

            for g in range(6):
                rowmax(g, nc.vector)
                sub_g(g, sub_eng[sub_cfg[g]])
                sq_g(g)
                if g == 2:
                    head_chunk(0)   # planes 0..31 complete after g2 rowmax
                elif g == 5:
                    head_chunk(1)   # planes 32..63 complete after g5 rowmax
                diag(g)

            # groups 6/7: follow the split loads; rowmax on DVE (GPSIMD has
            # no free-axis reduce), subtract on GPSIMD
            for k0, k1 in KSPLITS:
                rowmax(6, nc.vector, k0, k1)
                rowmax(7, nc.vector, k0, k1)
                sub_g(6, sub_eng[sub_cfg[6]], k0, k1)
                sq_g(6, k0, k1)
                sub_g(7, sub_eng[sub_cfg[7]], k0, k1)
                sq_g(7, k0, k1)
            ridu_inst = head_chunk(2)   # planes 64..87

            gath = sb.tile([PL, 128], f32, name="gath")
            if "gzero" in skips:
                nc.vector.memset(ridu[:], 0)
            nc.gpsimd.indirect_dma_start(
                out=gath[:],
                out_offset=None,
                in_=cp.rearrange("b s c h w -> (b s c h) w"),
                in_offset=bass.IndirectOffsetOnAxis(ap=ridu[:, 0:1], axis=0),
            )

            # x-side tail ops (need xf only) before the diags
            u = sb.tile([PL, 1], f32, name="u")
            nc.vector.scalar_tensor_tensor(
                out=u[:], in0=xf[:], scalar=pred9[:, 7:8], in1=t1[:],
                op0=Alu.add, op1=Alu.subtract,
            )
            w1 = sb.tile([PL, 1], f32, name="w1")
            nc.vector.scalar_tensor_tensor(
                out=w1[:], in0=u[:], scalar=u[:, 0:1], in1=cls[:],
                op0=Alu.mult, op1=Alu.add,
            )

            # fill the gather window with the last two diags
            d6 = diag(6)
            d7 = diag(7)
            tile.add_dep_helper(
                d6.ins, ridu_inst.ins, sync=False, reason="defer diag past head"
            )

            # ---- y from the gathered rows ----
            ysc = scr.tile([PL, 128], f32, name="ysc", tag="xysc")
            yf = sb.tile([PL, 1], f32, name="yf")
            nc.vector.scalar_tensor_tensor(
                out=ysc[:], in0=gath[:], scalar=Mv[:, 0:1], in1=io_t[0:PL, :],
                op0=Alu.is_equal, op1=Alu.mult, accum_out=yf[:],
            )

            conf = sb.tile([PL, 1], f32, name="conf")
            nc.scalar.activation(
                out=conf[:], in_=Mv[:], func=Act.Square, bias=1.0, scale=-1.0
            )

            # v = (yf+p8)-t3; w2 = v*v + w1; perkp = (w2+conf)*vv
            v = sb.tile([PL, 1], f32, name="v")
            nc.vector.scalar_tensor_tensor(
                out=v[:], in0=yf[:], scalar=pred9[:, 8:9], in1=t3[:],
                op0=Alu.add, op1=Alu.subtract,
            )
            w2 = sb.tile([PL, 1], f32, name="w2")
            nc.vector.scalar_tensor_tensor(
                out=w2[:], in0=v[:], scalar=v[:, 0:1], in1=w1[:],
                op0=Alu.mult, op1=Alu.add,
            )
            perkp = sb.tile([PL, 1], f32, name="perkp")
            nc.vector.scalar_tensor_tensor(
                out=perkp[:], in0=w2[:], scalar=conf[:, 0:1], in1=vv[:],
                op0=Alu.add, op1=Alu.mult,
            )

            # ---- final reductions into ONE PSUM bank, single out DMA ----
            psum_out = ps.tile([1, 2 * G], f32, name="psum_out", tag="fin")
            nc.tensor.matmul(
                out=psum_out[:, 0:G], lhsT=on_t[:], rhs=hsum[:],
                start=True, stop=False, skip_group_check=True,
            )
            nc.tensor.matmul(
                out=psum_out[:, G : 2 * G], lhsT=perkp[:], rhs=km_t[:],
                start=False, stop=True, skip_group_check=True,
            )
            out_row = sb.tile([1, 2 * G], f32, name="out_row")
            nc.vector.tensor_copy(out=out_row[:], in_=psum_out[:])
            nc.sync.dma_start(out_all, out_row[:])

        if loop_n > 1:
            # on-device timing loop: each iteration is separated by the
            # For_i back-edge barrier, so wall time ~= N * (span + ~2us)
            with tc.For_i(0, loop_n, 1):
                emit()
        else:
            for _ in range(reps):
                emit()

    nc.compile()
    return nc


def _get_nc(reps=1, loop_n=1):
    key = f"nc{reps}_{loop_n}"
    if key not in _CACHE:
        _CACHE[key] = _build_module(reps, loop_n)
    return _CACHE[key]


def _in_maps(combined_preds, heatmaps, labels):
    cp = np.ascontiguousarray(combined_preds, dtype=np.float32)
    hmr = np.ascontiguousarray(heatmaps, dtype=np.float32)
    lb = np.ascontiguousarray(labels, dtype=np.float32)
    maps = []
    for i in range(NCORES):
        b0 = BL * i
        maps.append(
            {
                "cp": np.ascontiguousarray(cp[b0 : b0 + BL]),
                "hmr": np.ascontiguousarray(hmr[b0 : b0 + BL]),
                "lbl": np.ascontiguousarray(lb[b0 : b0 + BL]),
            }
        )
    return maps


def run(combined_preds, heatmaps, labels, trace=False):
    """Run on hardware; returns ((heat, label), BassKernelResults)."""
    from concourse import bass_utils

    nc = _get_nc()
    res = bass_utils.run_bass_kernel_spmd(
        nc,
        _in_maps(combined_preds, heatmaps, labels),
        core_ids=list(range(NCORES)),
        trace=trace,
    )
    heat = np.concatenate(
        [res.results[i]["out_all"][:, 0:G].reshape(BL, S) for i in range(NCORES)],
        axis=0,
    )
    lab = np.concatenate(
        [res.results[i]["out_all"][:, G : 2 * G].reshape(BL, S) for i in range(NCORES)],
        axis=0,
    )
    return (heat, lab), res


def kernel(combined_preds, heatmaps, labels):
    (heat, lab), _ = run(combined_preds, heatmaps, labels)
    return heat, lab
